# revision 38
# baseline (speedup 1.0000x reference)
"""Distributed multi-head attention kernel for 8 TRN2 NeuronCores.

Problem: x [4, 2048, 1024] -> qkv proj -> 16-head attention (d=64)
         -> out proj + bias -> [4, 2048, 1024].

Sharding (head-split, no collectives): core i handles batch b = i//2 and
head-half hh = i%2 (8 heads, full 2048-token sequence). Each core
computes Q/K/V projections only for its own 8 heads, attention for
those heads, and a partial output projection (+bias on hh=0 cores).
The host sums the two partial outputs per batch.

The kernel is paced by the ScalarE exp() floor (256 ACTIVATEs of
[128,1024] psum spans at ~1.34us each) and arranged so neither the PE
nor the in-order engine queues ever stall that cadence:

  - S^T spans are double-buffered (4 psum banks) and emitted two steps
    ahead; after each ACT the S lookahead is emitted BEFORE PV so the
    ACT-critical chain (S -> ACT) stays short.
  - PV runs on a retimed schedule (lag 4 at the start of each unit,
    catching up with two double-PV steps mid-unit) so the previous
    unit's U drain gets a ~3-step window before its single psum buffer
    is needed again -- the in-order PE queue never blocks on the DVE
    drain (which previously caused HAM re-throttle windows).
  - U [65,1024] accumulates PV per (head, q-chunk); row 64 (ones column
    of V) gives softmax denominators free. At drain, 1/D is computed by
    DVE reciprocal straight from the psum row, and the normalize runs
    lazily on GpSimd (partition_broadcast + multiply) with no PE or
    extra DVE work.
  - Projections and the output projection run as small filler closures
    (<=4 matmuls) paced by emission deadlines between attention steps.
    Deadlines are also correctness-critical: a tile-writing filler must
    be EMITTED before any reader (the Tile framework only orders reads
    against earlier-emitted writes), so emit_S/PV force-pop exactly the
    closures they depend on via per-(tensor,pair,chunk) counters.
  - Input DMAs are split across the two hardware DGE queues (SP + ACT)
    in first-need order (wk+xt tokens 0-1023 on SP; wq/wv early on ACT)
    so the first projections start ~6us in; a short dense warmup keeps
    the PE HAM clock-gate warm while they land.
  - Unit order is qc-major within a head-pair so the first token-half's
    output projection + DMA-out streams during the last units.

Known environment hazard: the package power/thermal limiter can pin the
PE HAM gate at K=4/8 (1.2 GHz) for ~100us stretches when all 8 cores
run hot; the schedule stays correct and near-optimal in either clock
regime.
"""

import numpy as np
import ml_dtypes

B = 4
N = 2048
DIM = 1024
HEADS = 16
DH = 64
NCORES = 8
NH = 8       # heads per core
NPAIR = 4    # head pairs per core

WARMUP_MM = 20   # dense PE warmup matmuls during the input-DMA window
# PV step offset within a unit, per kc (lag 4 at unit start gives the
# previous unit's U drain a 3-step window and relaxes V-projection
# deadlines in unit 0; doubles at +12/+13 catch up to lag 2 by the end).
PV_REL = [4, 5, 6, 7, 8, 9, 10, 11, 12, 12, 13, 13, 14, 15, 16, 17]
USE_GPSIMD_BCAST = True

_CACHE = {}


def _build_nc():
    from contextlib import ExitStack

    import concourse.bass as bass
    import concourse.mybir as mybir
    import concourse.tile as tile
    from concourse import bacc

    f32 = mybir.dt.float32
    bf16 = mybir.dt.bfloat16
    f16 = mybir.dt.float16
    EXP = mybir.ActivationFunctionType.Exp

    nc = bacc.Bacc("TRN2", target_bir_lowering=False, debug=False,
                   num_devices=NCORES)

    xt_d = nc.dram_tensor("xt", [DIM, N], bf16, kind="ExternalInput")
    wq_d = nc.dram_tensor("wq", [128, 4096], bf16, kind="ExternalInput")
    wk_d = nc.dram_tensor("wk", [128, 4096], bf16, kind="ExternalInput")
    wv_d = nc.dram_tensor("wv", [128, 4096], bf16, kind="ExternalInput")
    wo_d = nc.dram_tensor("wo", [NPAIR, 128, DIM], bf16, kind="ExternalInput")
    bias_d = nc.dram_tensor("bias", [128, DIM], bf16, kind="ExternalInput")
    out_d = nc.dram_tensor("out", [N, DIM], bf16, kind="ExternalOutput")

    with tile.TileContext(nc) as tc, ExitStack() as top:
        const_pool = top.enter_context(tc.tile_pool(name="const", bufs=1))
        s_ps = top.enter_context(tc.tile_pool(name="sps", bufs=2, space="PSUM"))
        u_ps = top.enter_context(tc.tile_pool(name="ups", bufs=1, space="PSUM"))
        mm_ps = top.enter_context(tc.tile_pool(name="mmps", bufs=2, space="PSUM"))
        es_pool = top.enter_context(tc.tile_pool(name="es", bufs=6))
        ur_pool = top.enter_context(tc.tile_pool(name="ur", bufs=4))
        d_pool = top.enter_context(tc.tile_pool(name="dsb", bufs=4))
        r_pool = top.enter_context(tc.tile_pool(name="rsb", bufs=2))
        un_pool = top.enter_context(tc.tile_pool(name="un", bufs=1))

        ones_t = const_pool.tile([1, 128], f16, tag="ones", name="ones")
        nc.gpsimd.memset(ones_t[:], 1.0)
        ones32_t = const_pool.tile([1, 128], f32, tag="ones32", name="ones32")
        nc.gpsimd.memset(ones32_t[:], 1.0)
        warm_t = const_pool.tile([128, 512], bf16, tag="warm", name="warm")
        nc.gpsimd.memset(warm_t[:], 0.0)
        bias_t = const_pool.tile([128, DIM], bf16, tag="bias", name="bias")

        # ---- static input tiles -------------------------------------
        # (xt/w innermost: released mid-kernel; pool releases are LIFO)
        qkv_pool = tc.alloc_tile_pool(name="qkv", bufs=1)
        wo_pool = tc.alloc_tile_pool(name="wo", bufs=1)
        xt_pool = tc.alloc_tile_pool(name="xt", bufs=1)
        w_pool = tc.alloc_tile_pool(name="w", bufs=1)
        xt_all = xt_pool.tile([128, 8, N], bf16, tag="xt", name="xt")
        xt = [xt_all[:, i, :] for i in range(8)]
        # wq/wk are PAIR-MAJOR: pair p's 8 fc-chunks of 128 cols live at
        # cols [p*1024, (p+1)*1024) -- so pair 0's weights (0.25MB) can
        # DMA first and the first projections start ~4x earlier. wv
        # stays fc-major (v units use all heads of an fc chunk at once).
        wq_t = w_pool.tile([128, 4096], bf16, tag="wq", name="wq")
        wk_t = w_pool.tile([128, 4096], bf16, tag="wk", name="wk")
        wv_t = w_pool.tile([128, 4096], bf16, tag="wv", name="wv")
        wv = [wv_t[:, i * 512:(i + 1) * 512] for i in range(8)]

        def w_pk(w_t, p, fc):
            base = p * 1024 + fc * 128
            return w_t[:, base:base + 128]
        WO = [wo_pool.tile([128, DIM], bf16, tag=f"wo{p}", name=f"wo{p}")
              for p in range(NPAIR)]

        # DMA split across the two HWDGE queues in first-need order.
        # xt moves as per-slot [128, 512] transfers (slot i = dram rows
        # i*128..i*128+127): per-partition contiguous 1KB lines -- the
        # old all-slot rearrange was a strided gather that ran at ~85
        # GB/s (12us for the first quarter) and delayed the first ACT.
        # SP: wk + xt slots 0-3 (by column quarter), then wo/bias
        #     (needed ~2/3 in).
        # ACT: wq + xt slots 4-7 quarters 0-1, wv, xt slots 4-7 rest.
        xt_src = xt_d.ap().rearrange("(i p) n -> i p n", p=128)

        def dma_w_pair(eng, w_t, w_d, p):
            eng.dma_start(w_t[:, p * 1024:(p + 1) * 1024],
                          w_d.ap()[:, p * 1024:(p + 1) * 1024])

        # Engine-queue discipline: a dma_start is a descriptor-issue on
        # that ENGINE's queue, and the ~4-deep per-engine semaphore
        # window makes the 5th+ issue BLOCK the engine until transfers
        # drain. ScalarE paces the ACTs, VectorE/GpSimd have early work
        # -- so each of those gets at most 4 wait-free issues (critical
        # set only), and everything else queues on Sync, which has
        # nothing better to do until the tail.
        def dma_xt(eng, sl, qtr):
            eng.dma_start(xt_all[:, sl, qtr * 512:(qtr + 1) * 512],
                          xt_src[sl, :, qtr * 512:(qtr + 1) * 512])

        # critical set: wk-p0/wq-p0 + xt quarters 0-1 (first ACT gate).
        # Only SP/Activation/GpSimd can initiate DMAs; ScalarE gets just
        # 4 issues (it paces the ACTs), GpSimd bulk-carries quarter 1.
        dma_w_pair(nc.sync, wk_t, wk_d, 0)
        for sl in range(3):
            dma_xt(nc.sync, sl, 0)
        dma_w_pair(nc.scalar, wq_t, wq_d, 0)
        for sl in range(3, 6):
            dma_xt(nc.scalar, sl, 0)
        for sl in range(6, 8):
            dma_xt(nc.gpsimd, sl, 0)
        for sl in range(0, 6):
            dma_xt(nc.gpsimd, sl, 1)
        for sl in range(6, 8):
            dma_xt(nc.sync, sl, 1)

        # everything else on Sync in need-order: wv (V fillers from
        # ~step 0), xt quarters 2-3 (K/Q t=2,3 from step ~4), remaining
        # w pairs (from step ~34), wo/bias (from step ~160).
        nc.sync.dma_start(wv_t[:], wv_d.ap()[:])
        for qtr in range(2, 4):
            for sl in range(8):
                dma_xt(nc.sync, sl, qtr)
        for p in range(1, NPAIR):
            dma_w_pair(nc.sync, wk_t, wk_d, p)
            dma_w_pair(nc.sync, wq_t, wq_d, p)
        for p in range(NPAIR):
            nc.sync.dma_start(WO[p][:], wo_d.ap()[p])
        nc.sync.dma_start(bias_t[:], bias_d.ap()[:])

        QT = [qkv_pool.tile([128, N], bf16, tag=f"q{p}", name=f"q{p}")
              for p in range(NPAIR)]
        KT = [qkv_pool.tile([128, N], bf16, tag=f"k{p}", name=f"k{p}")
              for p in range(NPAIR)]
        VT = [qkv_pool.tile([128, NH, 65], bf16, tag=f"v{tb}", name=f"v{tb}")
              for tb in range(16)]
        UN = [un_pool.tile([128, N], bf16, tag=f"un{p}", name=f"un{p}")
              for p in range(NPAIR)]

        # ---- PE warmup: lift HAM to K=8/8 while input DMAs land -----
        # dense K=128 matmuls: K=1 streams don't register enough activity
        # to lift the HAM clock gate. Sized to end roughly when the first
        # projection inputs land (~6us). Also preload the exp table set
        # (~2.7us ACT_TABLE_LOAD) with a dummy activation so the first
        # real ACT doesn't pay for it.
        dummy_es = const_pool.tile([1, 8], bf16, tag="dummy_es", name="de")
        for i in range(WARMUP_MM):
            ps = mm_ps.tile([128, 512], f32, tag="mm", name="wu")
            nc.tensor.matmul(ps[:], warm_t[:, 0:128], warm_t[:],
                             start=True, stop=True)
            if i == 0:
                nc.scalar.activation(dummy_es[:], warm_t[0:1, 0:8], EXP,
                                     scale=0.125)

        # ---- projection unit closures (split into <=4-MM halves) ----
        # pending counters are per (tensor, pair, tchunk) so a force only
        # pulls exactly what an S step needs, not the whole pair.
        pending_kq = {}
        pending_v = {tb: 0 for tb in range(16)}

        def kq_first(box, w, p, t):
            ps = mm_ps.tile([128, 512], f32, tag="mm", name="mm")
            box[0] = ps
            for fc in range(4):
                nc.tensor.matmul(
                    ps[:], w_pk(w, p, fc),
                    xt[fc][:, t * 512:(t + 1) * 512],
                    start=(fc == 0), stop=False)

        def kq_second(box, dest, w, p, t):
            ps = box[0]
            for fc in range(4, 8):
                nc.tensor.matmul(
                    ps[:], w_pk(w, p, fc),
                    xt[fc][:, t * 512:(t + 1) * 512],
                    start=False, stop=(fc == 7))
            nc.vector.tensor_copy(dest[p][:, t * 512:(t + 1) * 512], ps[:])

        def kq_unit(dest, w, p, t):
            box = [None]
            kq_first(box, w, p, t)
            kq_second(box, dest, w, p, t)

        def v_first(box, tb):
            ps = mm_ps.tile([128, 512], f32, tag="mm", name="mm")
            box[0] = ps
            for fc in range(4):
                nc.tensor.matmul(
                    ps[:], xt[fc][:, tb * 128:(tb + 1) * 128], wv[fc][:],
                    start=(fc == 0), stop=False)

        def v_second(box, tb):
            ps = box[0]
            for fc in range(4, 8):
                nc.tensor.matmul(
                    ps[:], xt[fc][:, tb * 128:(tb + 1) * 128], wv[fc][:],
                    start=False, stop=(fc == 7))
            nc.vector.tensor_copy(
                VT[tb][:, :, 0:64],
                ps[:].rearrange("p (h d) -> p h d", d=64))
            nc.gpsimd.memset(VT[tb][:, :, 64:65], 1.0)

        def v_unit(tb):
            box = [None]
            v_first(box, tb)
            v_second(box, tb)

        # ---- fillers with emission deadlines ------------------------
        fillers = []
        state = {"emitted": 0, "total": 0}

        def add_filler(latest, fn):
            fillers.append((latest, fn))

        def pop_filler():
            _, fn = fillers.pop(0)
            fn()
            state["emitted"] += 1

        def add_kq_filler(latest, dest, w, p, t):
            box = [None]
            key = (id(dest), p, t)
            pending_kq[key] = pending_kq.get(key, 0) + 2

            def first():
                kq_first(box, w, p, t)
                pending_kq[key] -= 1

            def second():
                kq_second(box, dest, w, p, t)
                pending_kq[key] -= 1

            # first+second MUST pop adjacently: the psum accumulation
            # tile in `box` is only protected against mm_ps pool reuse
            # once its reader (second) has been emitted.
            add_filler(latest, first)
            add_filler(latest, second)

        def add_v_filler(latest, tb):
            box = [None]
            pending_v[tb] += 2

            def first():
                v_first(box, tb)
                pending_v[tb] -= 1

            def second():
                v_second(box, tb)
                pending_v[tb] -= 1

            add_filler(latest, first)
            add_filler(latest, second)

        def maybe_fill(done, steps):
            # deadlines are correctness-critical (a write filler emitted
            # after its reader leaves the reader on stale data). Pop ONLY
            # the due entries, preserving queue order among them — a due
            # entry deep in the queue must NOT drag every earlier
            # not-yet-due entry with it (that bulk pop concentrated ~150
            # projection matmuls into one step and starved ACT for
            # ~50us). Cross-filler dependencies (norm -> passA -> passB)
            # are safe: their deadlines are ordered the same way as their
            # queue positions.
            i = 0
            while i < len(fillers):
                latest, _ = fillers[i]
                if latest is not None and done >= latest:
                    _, fn = fillers.pop(i)
                    fn()
                    state["emitted"] += 1
                else:
                    i += 1

        def force_keys(keys):
            while any(pending_kq.get(k, 0) > 0 for k in keys):
                pop_filler()

        def force_v(tb):
            while pending_v[tb] > 0:
                pop_filler()

        # preamble: what S(0)/S(1) need before the first ACT (V tiles
        # come later as fillers -- the first PV is retimed to step +4).
        # The j=0 halves of S(0)/S(1) only need Q chunk 0, so they are
        # emitted BEFORE the Q chunk-1 projection and the first two ACTs
        # run as 512-col halves -- exp starts ~3.5us earlier than
        # waiting for the full Q(0,0..1) chain.
        kq_unit(KT, wk_t, 0, 0)
        kq_unit(QT, wq_t, 0, 0)

        def emit_S_half(st, kc, j):
            nc.tensor.matmul(
                st[:, j * 512:(j + 1) * 512],
                KT[0][0:64, kc * 128:(kc + 1) * 128],
                QT[0][0:64, j * 512:(j + 1) * 512],
                start=True, stop=True)

        st0 = s_ps.tile([128, 1024], f32, tag="s", name="s")
        st1 = s_ps.tile([128, 1024], f32, tag="s", name="s")
        emit_S_half(st0, 0, 0)
        emit_S_half(st1, 1, 0)
        kq_unit(QT, wq_t, 0, 1)
        emit_S_half(st0, 0, 1)
        emit_S_half(st1, 1, 1)

        # remaining proj as deadline fillers, spread to land shortly
        # before their true need-times (selective popping honors these
        # exactly, ~4 matmuls per due step). Pair 3's deadlines are
        # capped at 157 so every xt/w read is EMITTED before the xt/w
        # pool release at u==9 (step ~161) -- the fin tiles reuse that
        # SBUF region and only emission order protects them.
        add_kq_filler(0, KT, wk_t, 0, 1)
        for tb in range(0, 16):
            add_v_filler(max(0, PV_REL[tb] - 4), tb)
        add_kq_filler(4, KT, wk_t, 0, 2)
        add_kq_filler(8, KT, wk_t, 0, 3)
        add_kq_filler(14, QT, wq_t, 0, 2)
        add_kq_filler(18, QT, wq_t, 0, 3)
        # pairs 1-3 spread UNIFORMLY over steps 22..150 (~1.6 MM/step):
        # clumping them near their need-times overloads those steps and
        # stalls the ACT cadence. All dues stay < the step-161 xt/w
        # release (see above) and ahead of every need-time.
        KQ_BASE = {1: (22, 5), 2: (62, 6), 3: (108, 6)}
        for p in range(1, NPAIR):
            base, step = KQ_BASE[p]
            for i, (dest, w, t) in enumerate(
                    [(KT, wk_t, t) for t in range(4)] +
                    [(QT, wq_t, t) for t in range(4)]):
                add_kq_filler(base + step * i, dest, w, p, t)

        # ---- attention, software-pipelined across all 16 units ------
        # qc-major within a pair: both heads' qc=0 first, so the first
        # token-half's output projection + DMA-out streams during the
        # last units.
        units = [(p, hh, qc) for p in range(NPAIR) for qc in range(2)
                 for hh in range(2)]
        NU = len(units)
        GTOT = NU * 16

        # PV retimed schedule: step -> list of global pv indices
        pv_at = {}
        for u in range(NU):
            for kc in range(16):
                pv_at.setdefault(u * 16 + PV_REL[kc], []).append(u * 16 + kc)
        LAST_STEP = max(pv_at)

        def emit_S(gidx):
            u, kc = divmod(gidx, 16)
            p, hh, qc = units[u]
            force_keys([(id(KT), p, kc // 4),
                        (id(QT), p, 2 * qc), (id(QT), p, 2 * qc + 1)])
            hb = hh * 64
            st = s_ps.tile([128, 1024], f32, tag="s", name="s")
            for j in range(2):
                nc.tensor.matmul(
                    st[:, j * 512:(j + 1) * 512],
                    KT[p][hb:hb + 64, kc * 128:(kc + 1) * 128],
                    QT[p][hb:hb + 64,
                          qc * 1024 + j * 512:qc * 1024 + j * 512 + 512],
                    start=True, stop=True)
            return st

        def norm_rest(p, hh, qc, ur, dsb, on_dve=False):
            """Lazy normalize: reciprocal of D (SBUF), broadcast, multiply.

            Steady state runs the broadcast+multiply on GpSimd (PE and
            DVE are the pacing engines there). The last two units use
            `on_dve`: PE K=1 broadcast into psum + DVE multiply reading
            the psum operand directly -- the GpSimd queue's dispatch and
            drain latency (~10us) would otherwise sit on the tail
            critical path gating the final out-projection.
            """
            hb = hh * 64
            rd = d_pool.tile([1, 1024], f32, tag="rd", name="rd")
            nc.vector.reciprocal_approx_fast(rd[:], dsb[:])
            if on_dve:
                for j in range(2):
                    bc = mm_ps.tile([128, 512], f32, tag="mm", name="bc")
                    nc.tensor.matmul(bc[0:64, :], ones32_t[:, 0:64],
                                     rd[:, j * 512:(j + 1) * 512],
                                     start=True, stop=True)
                    nc.vector.tensor_mul(
                        UN[p][hb:hb + 64,
                              qc * 1024 + j * 512:qc * 1024 + j * 512 + 512],
                        ur[:, j * 512:(j + 1) * 512], bc[0:64, :])
            elif USE_GPSIMD_BCAST:
                rsb = r_pool.tile([64, 1024], f32, tag="rsb", name="rsb")
                nc.gpsimd.partition_broadcast(rsb[:], rd[:], channels=64)
                nc.gpsimd.tensor_mul(
                    UN[p][hb:hb + 64, qc * 1024:(qc + 1) * 1024],
                    ur[:], rsb[:])
            else:
                # PE broadcast of 1/D (K=1 matmul), then multiply on GpSimd
                for j in range(2):
                    bc = mm_ps.tile([128, 512], f32, tag="mm", name="bc")
                    nc.tensor.matmul(bc[:], ones32_t[:],
                                     rd[:, j * 512:(j + 1) * 512],
                                     start=True, stop=True)
                    rsb = r_pool.tile([64, 1024], f32, tag="rsb", name="rsb")
                    nc.vector.tensor_copy(rsb[:, 0:512], bc[0:64, :])
                    nc.gpsimd.tensor_mul(
                        UN[p][hb:hb + 64,
                              qc * 1024 + j * 512:qc * 1024 + j * 512 + 512],
                        ur[:, j * 512:(j + 1) * 512], rsb[:, 0:512])

        S_tiles = {0: st0, 1: st1}
        U_box = [None]

        passA_added = [False, False]
        passB_added = [False]
        fin_state = {}

        def setup_fin():
            w_pool.release()
            xt_pool.release()
            fin_state["pool"] = tc.alloc_tile_pool(name="fin", bufs=1)
            fin_state["FIN"] = [
                fin_state["pool"].tile([128, DIM], bf16, tag=f"fin{qf}",
                                       name=f"fin{qf}")
                for qf in range(16)]

        def passA(qf, of):
            FIN = fin_state["FIN"]
            ps = mm_ps.tile([128, 512], f32, tag="mm", name="pa")
            for p in range(3):
                nc.tensor.matmul(
                    ps[:], UN[p][:, qf * 128:(qf + 1) * 128],
                    WO[p][:, of * 512:(of + 1) * 512],
                    start=(p == 0), stop=(p == 2))
            nc.vector.tensor_add(
                FIN[qf][:, of * 512:(of + 1) * 512], ps[:],
                bias_t[:, of * 512:(of + 1) * 512])

        def passB_of(qf, of):
            # add pair 3 onto the resident partial and stream that
            # column-half out immediately (don't wait for the full row).
            FIN = fin_state["FIN"]
            ps = mm_ps.tile([128, 512], f32, tag="mm", name="pb")
            nc.tensor.matmul(
                ps[:], UN[3][:, qf * 128:(qf + 1) * 128],
                WO[3][:, of * 512:(of + 1) * 512],
                start=True, stop=True)
            nc.vector.tensor_add(
                FIN[qf][:, of * 512:(of + 1) * 512],
                FIN[qf][:, of * 512:(of + 1) * 512], ps[:])
            # tail out-DMAs split across both HWDGE queues; in-loop ones
            # stay off the ScalarE queue (it paces the ACTs)
            dma_eng = nc.scalar if (qf >= 8 and of == 1) else nc.sync
            dma_eng.dma_start(
                out_d.ap()[qf * 128:(qf + 1) * 128,
                           of * 512:(of + 1) * 512],
                FIN[qf][:, of * 512:(of + 1) * 512])

        def passB(qf):
            passB_of(qf, 0)
            passB_of(qf, 1)

        es_tiles = {}

        def pv_job(gp, gnow):
            """PV for step gp (retimed: late enough that a late V tile or
            a pending U drain never blocks the in-order PE queue)."""
            u, kc = divmod(gp, 16)
            p, hh, qc = units[u]
            hloc = 2 * p + hh
            es = es_tiles.pop(gp)
            if kc == 0:
                U_box[0] = u_ps.tile([65, 1024], f32, tag="u", name="u")
            U = U_box[0]
            force_v(kc)
            for j in range(2):
                nc.tensor.matmul(
                    U[:, j * 512:(j + 1) * 512],
                    VT[kc][:, hloc, 0:65],
                    es[:, j * 512:(j + 1) * 512],
                    start=(kc == 0), stop=(kc == 15))
            if kc == 15:
                # fast U drain: D row first (it gates the lazy normalize),
                # then the U rows; frees U's single psum buffer inside the
                # 3-step boundary window.
                dsb = d_pool.tile([1, 1024], f32, tag="d", name="d")
                nc.vector.tensor_copy(dsb[:], U[64:65, :])
                ur = ur_pool.tile([64, 1024], bf16, tag="ur", name="ur")
                nc.vector.tensor_copy(ur[:], U[0:64, :])
                if u >= 14:
                    # last two units: normalize eagerly on PE+DVE so the
                    # tail passB isn't gated by the GpSimd queue.
                    norm_rest(p, hh, qc, ur, dsb, on_dve=True)
                    if u == 15:
                        # keep the PE HAM clock-gate warm across the
                        # norm->passB handoff (a >3.4us PE-idle window
                        # re-throttles to K=4/8 and runs the 32 tail
                        # matmuls at half clock)
                        for _ in range(6):
                            wps = mm_ps.tile([128, 512], f32, tag="mm",
                                             name="wu2")
                            nc.tensor.matmul(wps[:], warm_t[:, 0:128],
                                             warm_t[:], start=True, stop=True)
                else:
                    add_filler(min(gnow + 10, 250),
                               lambda p=p, hh=hh, qc=qc, ur=ur, dsb=dsb:
                               norm_rest(p, hh, qc, ur, dsb))
                if u == 9 and not passA_added[0]:
                    # pairs 0-2 qc0 done: out-proj for tokens 0-1023
                    passA_added[0] = True
                    setup_fin()
                    for i, (qf, of) in enumerate(
                            (qf, of) for qf in range(8) for of in range(2)):
                        add_filler(gnow + 12 + i * 3,
                                   lambda qf=qf, of=of: passA(qf, of))
                if u == 11 and not passA_added[1]:
                    passA_added[1] = True
                    for i, (qf, of) in enumerate(
                            (qf, of) for qf in range(8, 16) for of in range(2)):
                        add_filler(gnow + 12 + i * 2,
                                   lambda qf=qf, of=of: passA(qf, of))
                if u == 13 and not passB_added[0]:
                    passB_added[0] = True
                    i = 0
                    for qf in range(8):
                        for of in range(2):
                            add_filler(gnow + 12 + i,
                                       lambda qf=qf, of=of: passB_of(qf, of))
                            i += 1

        for gidx in range(LAST_STEP + 1):
            if gidx < GTOT:
                st = S_tiles.pop(gidx)
                es = es_pool.tile([128, 1024], bf16, tag="es", name="es")
                if gidx < 2:
                    # halves: the j=0 ACT only depends on the early j=0
                    # S matmul, not on the Q chunk-1 projection
                    for j in range(2):
                        nc.scalar.activation(
                            es[:, j * 512:(j + 1) * 512],
                            st[:, j * 512:(j + 1) * 512], EXP, scale=0.125)
                else:
                    nc.scalar.activation(es[:], st[:], EXP, scale=0.125)
                es_tiles[gidx] = es
            # PV before the S lookahead: S(g+2)'s first matmul carries a
            # write-after-read wait on ACT(g)'s psum buffer, and the
            # in-order PE queue would stall on it with ready PV work
            # parked behind. Fillers AFTER S: in heavy steps (the early
            # V crunch) a 10+-matmul filler burst ahead of S would
            # starve the ACT chain instead.
            for gp in pv_at.get(gidx, ()):
                pv_job(gp, gidx)
            if gidx + 2 < GTOT:
                S_tiles[gidx + 2] = emit_S(gidx + 2)
            maybe_fill(gidx, GTOT)

        # flush remaining fillers (incl. last norms and any passA/B)
        while fillers:
            pop_filler()

        # tail: second token-half out-proj + DMA
        for qf in range(8, 16):
            passB(qf)

        fin_state["pool"].release()
        wo_pool.release()
        qkv_pool.release()

    nc.compile()
    return nc


def _get_nc():
    if "nc" not in _CACHE:
        _CACHE["nc"] = _build_nc()
    return _CACHE["nc"]


def _make_in_maps(x, w_qkv, w_out, b_out):
    bf = ml_dtypes.bfloat16

    def wslice(w, hh):
        # fc-major: [1024, 512] -> [128, 8, 512] (partition p holds
        # w[fc*128+p, :] at slot fc) -> [128, 4096]
        s = np.asarray(w[:, hh * 512:(hh + 1) * 512], np.float32)
        return np.ascontiguousarray(
            s.reshape(8, 128, 512).transpose(1, 0, 2).reshape(128, 4096)
        ).astype(bf)

    def wslice_pair(w, hh):
        # pair-major: cols [p*1024+fc*128 : +128] hold pair p's fc-chunk
        # (partition = contraction row within the chunk)
        s = np.asarray(w[:, hh * 512:(hh + 1) * 512], np.float32)
        return np.ascontiguousarray(
            s.reshape(8, 128, 4, 128).transpose(1, 2, 0, 3).reshape(128, 4096)
        ).astype(bf)

    xts = [np.ascontiguousarray(np.asarray(x[b], np.float32).T).astype(bf)
           for b in range(B)]
    wq_f = w_qkv[:, 0:1024]
    wk_f = w_qkv[:, 1024:2048]
    wv_f = w_qkv[:, 2048:3072]
    wo_f = np.asarray(w_out, np.float32)  # [1024 inner, 1024 out]
    bias_rep = np.broadcast_to(
        np.asarray(b_out, np.float32).reshape(1, DIM), (128, DIM))
    zeros = np.zeros((128, DIM), np.float32)
    in_maps = []
    for i in range(NCORES):
        b, hh = i // 2, i % 2
        wo_core = np.ascontiguousarray(
            wo_f[hh * 512:(hh + 1) * 512, :]).reshape(NPAIR, 128, DIM)
        in_maps.append({
            "xt": xts[b],
            "wq": wslice_pair(wq_f, hh),
            "wk": wslice_pair(wk_f, hh),
            "wv": wslice(wv_f, hh),
            "wo": wo_core.astype(bf),
            "bias": np.ascontiguousarray(
                (bias_rep if hh == 0 else zeros)).astype(bf),
        })
    return in_maps


def _assemble(results):
    out = np.empty((B, N, DIM), np.float32)
    for b in range(B):
        out[b] = (results[2 * b]["out"].astype(np.float32) +
                  results[2 * b + 1]["out"].astype(np.float32))
    return out


def run(x, w_qkv, w_out, b_out, trace=False):
    """Run the kernel; returns (output, BassKernelResults)."""
    from concourse.bass_utils import run_bass_kernel_spmd
    nc = _get_nc()
    in_maps = _make_in_maps(x, w_qkv, w_out, b_out)
    res = run_bass_kernel_spmd(nc, in_maps, core_ids=list(range(NCORES)),
                               trace=trace)
    return _assemble(res.results), res


def kernel(x, w_qkv, w_out, b_out):
    out, _ = run(x, w_qkv, w_out, b_out, trace=False)
    return out


# revision 39
# speedup vs baseline: 1.0160x; 1.0160x over previous
"""Distributed multi-head attention kernel for 8 TRN2 NeuronCores.

Problem: x [4, 2048, 1024] -> qkv proj -> 16-head attention (d=64)
         -> out proj + bias -> [4, 2048, 1024].

Sharding (head-split, no collectives): core i handles batch b = i//2 and
head-half hh = i%2 (8 heads, full 2048-token sequence). Each core
computes Q/K/V projections only for its own 8 heads, attention for
those heads, and a partial output projection (+bias on hh=0 cores).
The host sums the two partial outputs per batch.

The kernel is paced by the ScalarE exp() floor (256 ACTIVATEs of
[128,1024] psum spans at ~1.34us each) and arranged so neither the PE
nor the in-order engine queues ever stall that cadence:

  - S^T spans are double-buffered (4 psum banks) and emitted two steps
    ahead; after each ACT the S lookahead is emitted BEFORE PV so the
    ACT-critical chain (S -> ACT) stays short.
  - PV runs on a retimed schedule (lag 4 at the start of each unit,
    catching up with two double-PV steps mid-unit) so the previous
    unit's U drain gets a ~3-step window before its single psum buffer
    is needed again -- the in-order PE queue never blocks on the DVE
    drain (which previously caused HAM re-throttle windows).
  - U [65,1024] accumulates PV per (head, q-chunk); row 64 (ones column
    of V) gives softmax denominators free. At drain, 1/D is computed by
    DVE reciprocal straight from the psum row, and the normalize runs
    lazily on GpSimd (partition_broadcast + multiply) with no PE or
    extra DVE work.
  - Projections and the output projection run as small filler closures
    (<=4 matmuls) paced by emission deadlines between attention steps.
    Deadlines are also correctness-critical: a tile-writing filler must
    be EMITTED before any reader (the Tile framework only orders reads
    against earlier-emitted writes), so emit_S/PV force-pop exactly the
    closures they depend on via per-(tensor,pair,chunk) counters.
  - Input DMAs are split across the two hardware DGE queues (SP + ACT)
    in first-need order (wk+xt tokens 0-1023 on SP; wq/wv early on ACT)
    so the first projections start ~6us in; a short dense warmup keeps
    the PE HAM clock-gate warm while they land.
  - Unit order is qc-major within a head-pair so the first token-half's
    output projection + DMA-out streams during the last units.

Known environment hazard: the package power/thermal limiter can pin the
PE HAM gate at K=4/8 (1.2 GHz) for ~100us stretches when all 8 cores
run hot; the schedule stays correct and near-optimal in either clock
regime.
"""

import numpy as np
import ml_dtypes

B = 4
N = 2048
DIM = 1024
HEADS = 16
DH = 64
NCORES = 8
NH = 8       # heads per core
NPAIR = 4    # head pairs per core

WARMUP_MM = 20   # dense PE warmup matmuls during the input-DMA window
# PV step offset within a unit, per kc (lag 4 at unit start gives the
# previous unit's U drain a 3-step window and relaxes V-projection
# deadlines in unit 0; doubles at +12/+13 catch up to lag 2 by the end).
PV_REL = [4, 5, 6, 7, 8, 9, 10, 11, 12, 12, 13, 13, 14, 15, 16, 17]
USE_GPSIMD_BCAST = True

_CACHE = {}


def _build_nc():
    from contextlib import ExitStack

    import concourse.bass as bass
    import concourse.mybir as mybir
    import concourse.tile as tile
    from concourse import bacc

    f32 = mybir.dt.float32
    bf16 = mybir.dt.bfloat16
    f16 = mybir.dt.float16
    EXP = mybir.ActivationFunctionType.Exp

    nc = bacc.Bacc("TRN2", target_bir_lowering=False, debug=False,
                   num_devices=NCORES)

    xt_d = nc.dram_tensor("xt", [DIM, N], bf16, kind="ExternalInput")
    wq_d = nc.dram_tensor("wq", [128, 4096], bf16, kind="ExternalInput")
    wk_d = nc.dram_tensor("wk", [128, 4096], bf16, kind="ExternalInput")
    wv_d = nc.dram_tensor("wv", [128, 4096], bf16, kind="ExternalInput")
    wo_d = nc.dram_tensor("wo", [NPAIR, 128, DIM], bf16, kind="ExternalInput")
    bias_d = nc.dram_tensor("bias", [128, DIM], bf16, kind="ExternalInput")
    out_d = nc.dram_tensor("out", [N, DIM], bf16, kind="ExternalOutput")

    with tile.TileContext(nc) as tc, ExitStack() as top:
        const_pool = top.enter_context(tc.tile_pool(name="const", bufs=1))
        s_ps = top.enter_context(tc.tile_pool(name="sps", bufs=2, space="PSUM"))
        u_ps = top.enter_context(tc.tile_pool(name="ups", bufs=1, space="PSUM"))
        mm_ps = top.enter_context(tc.tile_pool(name="mmps", bufs=2, space="PSUM"))
        es_pool = top.enter_context(tc.tile_pool(name="es", bufs=7))
        ur_pool = top.enter_context(tc.tile_pool(name="ur", bufs=4))
        d_pool = top.enter_context(tc.tile_pool(name="dsb", bufs=4))
        r_pool = top.enter_context(tc.tile_pool(name="rsb", bufs=2))
        un_pool = top.enter_context(tc.tile_pool(name="un", bufs=1))

        ones_t = const_pool.tile([1, 128], f16, tag="ones", name="ones")
        nc.gpsimd.memset(ones_t[:], 1.0)
        ones32_t = const_pool.tile([1, 128], f32, tag="ones32", name="ones32")
        nc.gpsimd.memset(ones32_t[:], 1.0)
        warm_t = const_pool.tile([128, 512], bf16, tag="warm", name="warm")
        nc.gpsimd.memset(warm_t[:], 0.0)
        bias_t = const_pool.tile([128, DIM], bf16, tag="bias", name="bias")

        # ---- static input tiles -------------------------------------
        # (xt/w innermost: released mid-kernel; pool releases are LIFO)
        qkv_pool = tc.alloc_tile_pool(name="qkv", bufs=1)
        wo_pool = tc.alloc_tile_pool(name="wo", bufs=1)
        xt_pool = tc.alloc_tile_pool(name="xt", bufs=1)
        w_pool = tc.alloc_tile_pool(name="w", bufs=1)
        xt_all = xt_pool.tile([128, 8, N], bf16, tag="xt", name="xt")
        xt = [xt_all[:, i, :] for i in range(8)]
        # wq/wk are PAIR-MAJOR: pair p's 8 fc-chunks of 128 cols live at
        # cols [p*1024, (p+1)*1024) -- so pair 0's weights (0.25MB) can
        # DMA first and the first projections start ~4x earlier. wv
        # stays fc-major (v units use all heads of an fc chunk at once).
        wq_t = w_pool.tile([128, 4096], bf16, tag="wq", name="wq")
        wk_t = w_pool.tile([128, 4096], bf16, tag="wk", name="wk")
        wv_t = w_pool.tile([128, 4096], bf16, tag="wv", name="wv")
        wv = [wv_t[:, i * 512:(i + 1) * 512] for i in range(8)]

        def w_pk(w_t, p, fc):
            base = p * 1024 + fc * 128
            return w_t[:, base:base + 128]
        WO = [wo_pool.tile([128, DIM], bf16, tag=f"wo{p}", name=f"wo{p}")
              for p in range(NPAIR)]

        # DMA split across the two HWDGE queues in first-need order.
        # xt moves as per-slot [128, 512] transfers (slot i = dram rows
        # i*128..i*128+127): per-partition contiguous 1KB lines -- the
        # old all-slot rearrange was a strided gather that ran at ~85
        # GB/s (12us for the first quarter) and delayed the first ACT.
        # SP: wk + xt slots 0-3 (by column quarter), then wo/bias
        #     (needed ~2/3 in).
        # ACT: wq + xt slots 4-7 quarters 0-1, wv, xt slots 4-7 rest.
        xt_src = xt_d.ap().rearrange("(i p) n -> i p n", p=128)

        def dma_w_pair(eng, w_t, w_d, p):
            eng.dma_start(w_t[:, p * 1024:(p + 1) * 1024],
                          w_d.ap()[:, p * 1024:(p + 1) * 1024])

        # Engine-queue discipline: a dma_start is a descriptor-issue on
        # that ENGINE's queue, and the ~4-deep per-engine semaphore
        # window makes the 5th+ issue BLOCK the engine until transfers
        # drain. ScalarE paces the ACTs, VectorE/GpSimd have early work
        # -- so each of those gets at most 4 wait-free issues (critical
        # set only), and everything else queues on Sync, which has
        # nothing better to do until the tail.
        def dma_xt(eng, sl, qtr):
            eng.dma_start(xt_all[:, sl, qtr * 512:(qtr + 1) * 512],
                          xt_src[sl, :, qtr * 512:(qtr + 1) * 512])

        # critical set: wk-p0/wq-p0 + xt quarters 0-1 (first ACT gate).
        # Only SP/Activation/GpSimd can initiate DMAs; ScalarE gets just
        # 4 issues (it paces the ACTs), GpSimd bulk-carries quarter 1.
        dma_w_pair(nc.sync, wk_t, wk_d, 0)
        for sl in range(3):
            dma_xt(nc.sync, sl, 0)
        dma_w_pair(nc.scalar, wq_t, wq_d, 0)
        for sl in range(3, 6):
            dma_xt(nc.scalar, sl, 0)
        for sl in range(6, 8):
            dma_xt(nc.gpsimd, sl, 0)
        for sl in range(0, 6):
            dma_xt(nc.gpsimd, sl, 1)
        for sl in range(6, 8):
            dma_xt(nc.sync, sl, 1)

        # everything else on Sync in need-order: wv (V fillers from
        # ~step 0), xt quarters 2-3 (K/Q t=2,3 from step ~4), remaining
        # w pairs (from step ~34), wo/bias (from step ~160).
        nc.sync.dma_start(wv_t[:], wv_d.ap()[:])
        for qtr in range(2, 4):
            for sl in range(8):
                dma_xt(nc.sync, sl, qtr)
        for p in range(1, NPAIR):
            dma_w_pair(nc.sync, wk_t, wk_d, p)
            dma_w_pair(nc.sync, wq_t, wq_d, p)
        for p in range(NPAIR):
            nc.sync.dma_start(WO[p][:], wo_d.ap()[p])
        nc.sync.dma_start(bias_t[:], bias_d.ap()[:])

        QT = [qkv_pool.tile([128, N], bf16, tag=f"q{p}", name=f"q{p}")
              for p in range(NPAIR)]
        KT = [qkv_pool.tile([128, N], bf16, tag=f"k{p}", name=f"k{p}")
              for p in range(NPAIR)]
        VT = [qkv_pool.tile([128, NH, 65], bf16, tag=f"v{tb}", name=f"v{tb}")
              for tb in range(16)]
        UN = [un_pool.tile([128, N], bf16, tag=f"un{p}", name=f"un{p}")
              for p in range(NPAIR)]

        # ---- PE warmup: lift HAM to K=8/8 while input DMAs land -----
        # dense K=128 matmuls: K=1 streams don't register enough activity
        # to lift the HAM clock gate. Sized to end roughly when the first
        # projection inputs land (~6us). Also preload the exp table set
        # (~2.7us ACT_TABLE_LOAD) with a dummy activation so the first
        # real ACT doesn't pay for it.
        dummy_es = const_pool.tile([1, 8], bf16, tag="dummy_es", name="de")
        for i in range(WARMUP_MM):
            ps = mm_ps.tile([128, 512], f32, tag="mm", name="wu")
            nc.tensor.matmul(ps[:], warm_t[:, 0:128], warm_t[:],
                             start=True, stop=True)
            if i == 0:
                nc.scalar.activation(dummy_es[:], warm_t[0:1, 0:8], EXP,
                                     scale=0.125)

        # ---- projection unit closures (split into <=4-MM halves) ----
        # pending counters are per (tensor, pair, tchunk) so a force only
        # pulls exactly what an S step needs, not the whole pair.
        pending_kq = {}
        pending_v = {tb: 0 for tb in range(16)}

        def kq_first(box, w, p, t):
            ps = mm_ps.tile([128, 512], f32, tag="mm", name="mm")
            box[0] = ps
            for fc in range(4):
                nc.tensor.matmul(
                    ps[:], w_pk(w, p, fc),
                    xt[fc][:, t * 512:(t + 1) * 512],
                    start=(fc == 0), stop=False)

        def kq_second(box, dest, w, p, t):
            ps = box[0]
            for fc in range(4, 8):
                nc.tensor.matmul(
                    ps[:], w_pk(w, p, fc),
                    xt[fc][:, t * 512:(t + 1) * 512],
                    start=False, stop=(fc == 7))
            nc.vector.tensor_copy(dest[p][:, t * 512:(t + 1) * 512], ps[:])

        def kq_unit(dest, w, p, t):
            box = [None]
            kq_first(box, w, p, t)
            kq_second(box, dest, w, p, t)

        def v_first(box, tb):
            ps = mm_ps.tile([128, 512], f32, tag="mm", name="mm")
            box[0] = ps
            for fc in range(4):
                nc.tensor.matmul(
                    ps[:], xt[fc][:, tb * 128:(tb + 1) * 128], wv[fc][:],
                    start=(fc == 0), stop=False)

        def v_second(box, tb):
            ps = box[0]
            for fc in range(4, 8):
                nc.tensor.matmul(
                    ps[:], xt[fc][:, tb * 128:(tb + 1) * 128], wv[fc][:],
                    start=False, stop=(fc == 7))
            nc.vector.tensor_copy(
                VT[tb][:, :, 0:64],
                ps[:].rearrange("p (h d) -> p h d", d=64))
            nc.gpsimd.memset(VT[tb][:, :, 64:65], 1.0)

        def v_unit(tb):
            box = [None]
            v_first(box, tb)
            v_second(box, tb)

        # ---- fillers with emission deadlines ------------------------
        fillers = []
        state = {"emitted": 0, "total": 0}

        def add_filler(latest, fn):
            fillers.append((latest, fn))

        def pop_filler():
            _, fn = fillers.pop(0)
            fn()
            state["emitted"] += 1

        def add_kq_filler(latest, dest, w, p, t):
            box = [None]
            key = (id(dest), p, t)
            pending_kq[key] = pending_kq.get(key, 0) + 2

            def first():
                kq_first(box, w, p, t)
                pending_kq[key] -= 1

            def second():
                kq_second(box, dest, w, p, t)
                pending_kq[key] -= 1

            # first+second MUST pop adjacently: the psum accumulation
            # tile in `box` is only protected against mm_ps pool reuse
            # once its reader (second) has been emitted.
            add_filler(latest, first)
            add_filler(latest, second)

        def add_v_filler(latest, tb):
            box = [None]
            pending_v[tb] += 2

            def first():
                v_first(box, tb)
                pending_v[tb] -= 1

            def second():
                v_second(box, tb)
                pending_v[tb] -= 1

            add_filler(latest, first)
            add_filler(latest, second)

        def maybe_fill(done, steps):
            # deadlines are correctness-critical (a write filler emitted
            # after its reader leaves the reader on stale data). Pop ONLY
            # the due entries, preserving queue order among them — a due
            # entry deep in the queue must NOT drag every earlier
            # not-yet-due entry with it (that bulk pop concentrated ~150
            # projection matmuls into one step and starved ACT for
            # ~50us). Cross-filler dependencies (norm -> passA -> passB)
            # are safe: their deadlines are ordered the same way as their
            # queue positions.
            i = 0
            while i < len(fillers):
                latest, _ = fillers[i]
                if latest is not None and done >= latest:
                    _, fn = fillers.pop(i)
                    fn()
                    state["emitted"] += 1
                else:
                    i += 1

        def force_keys(keys):
            while any(pending_kq.get(k, 0) > 0 for k in keys):
                pop_filler()

        def force_v(tb):
            while pending_v[tb] > 0:
                pop_filler()

        # preamble: what S(0)/S(1) need before the first ACT (V tiles
        # come later as fillers -- the first PV is retimed to step +4).
        # The j=0 halves of S(0)/S(1) only need Q chunk 0, so they are
        # emitted BEFORE the Q chunk-1 projection and the first two ACTs
        # run as 512-col halves -- exp starts ~3.5us earlier than
        # waiting for the full Q(0,0..1) chain.
        kq_unit(KT, wk_t, 0, 0)
        kq_unit(QT, wq_t, 0, 0)

        def emit_S_half(st, kc, j):
            nc.tensor.matmul(
                st[:, j * 512:(j + 1) * 512],
                KT[0][0:64, kc * 128:(kc + 1) * 128],
                QT[0][0:64, j * 512:(j + 1) * 512],
                start=True, stop=True)

        st0 = s_ps.tile([128, 1024], f32, tag="s", name="s")
        st1 = s_ps.tile([128, 1024], f32, tag="s", name="s")
        emit_S_half(st0, 0, 0)
        emit_S_half(st1, 1, 0)
        kq_unit(QT, wq_t, 0, 1)
        emit_S_half(st0, 0, 1)
        emit_S_half(st1, 1, 1)

        # remaining proj as deadline fillers, spread to land shortly
        # before their true need-times (selective popping honors these
        # exactly, ~4 matmuls per due step). Pair 3's deadlines are
        # capped at 157 so every xt/w read is EMITTED before the xt/w
        # pool release at u==9 (step ~161) -- the fin tiles reuse that
        # SBUF region and only emission order protects them.
        add_kq_filler(0, KT, wk_t, 0, 1)
        for tb in range(0, 16):
            add_v_filler(max(0, PV_REL[tb] - 4), tb)
        add_kq_filler(4, KT, wk_t, 0, 2)
        add_kq_filler(8, KT, wk_t, 0, 3)
        add_kq_filler(14, QT, wq_t, 0, 2)
        add_kq_filler(18, QT, wq_t, 0, 3)
        # pairs 1-3 spread UNIFORMLY over steps 22..150 (~1.6 MM/step):
        # clumping them near their need-times overloads those steps and
        # stalls the ACT cadence. All dues stay < the step-161 xt/w
        # release (see above) and ahead of every need-time.
        KQ_BASE = {1: (22, 5), 2: (62, 6), 3: (108, 6)}
        for p in range(1, NPAIR):
            base, step = KQ_BASE[p]
            for i, (dest, w, t) in enumerate(
                    [(KT, wk_t, t) for t in range(4)] +
                    [(QT, wq_t, t) for t in range(4)]):
                add_kq_filler(base + step * i, dest, w, p, t)

        # ---- attention, software-pipelined across all 16 units ------
        # qc-major within a pair: both heads' qc=0 first, so the first
        # token-half's output projection + DMA-out streams during the
        # last units.
        units = [(p, hh, qc) for p in range(NPAIR) for qc in range(2)
                 for hh in range(2)]
        NU = len(units)
        GTOT = NU * 16

        # PV retimed schedule: step -> list of global pv indices
        pv_at = {}
        for u in range(NU):
            for kc in range(16):
                pv_at.setdefault(u * 16 + PV_REL[kc], []).append(u * 16 + kc)
        LAST_STEP = max(pv_at)

        def emit_S(gidx):
            u, kc = divmod(gidx, 16)
            p, hh, qc = units[u]
            force_keys([(id(KT), p, kc // 4),
                        (id(QT), p, 2 * qc), (id(QT), p, 2 * qc + 1)])
            hb = hh * 64
            st = s_ps.tile([128, 1024], f32, tag="s", name="s")
            for j in range(2):
                nc.tensor.matmul(
                    st[:, j * 512:(j + 1) * 512],
                    KT[p][hb:hb + 64, kc * 128:(kc + 1) * 128],
                    QT[p][hb:hb + 64,
                          qc * 1024 + j * 512:qc * 1024 + j * 512 + 512],
                    start=True, stop=True)
            return st

        def norm_rest(p, hh, qc, ur, dsb, on_dve=False):
            """Lazy normalize: reciprocal of D (SBUF), broadcast, multiply.

            Steady state runs the broadcast+multiply on GpSimd (PE and
            DVE are the pacing engines there). The last two units use
            `on_dve`: PE K=1 broadcast into psum + DVE multiply reading
            the psum operand directly -- the GpSimd queue's dispatch and
            drain latency (~10us) would otherwise sit on the tail
            critical path gating the final out-projection.
            """
            hb = hh * 64
            rd = d_pool.tile([1, 1024], f32, tag="rd", name="rd")
            nc.vector.reciprocal_approx_fast(rd[:], dsb[:])
            if on_dve:
                for j in range(2):
                    bc = mm_ps.tile([128, 512], f32, tag="mm", name="bc")
                    nc.tensor.matmul(bc[0:64, :], ones32_t[:, 0:64],
                                     rd[:, j * 512:(j + 1) * 512],
                                     start=True, stop=True)
                    nc.vector.tensor_mul(
                        UN[p][hb:hb + 64,
                              qc * 1024 + j * 512:qc * 1024 + j * 512 + 512],
                        ur[:, j * 512:(j + 1) * 512], bc[0:64, :])
            elif USE_GPSIMD_BCAST:
                rsb = r_pool.tile([64, 1024], f32, tag="rsb", name="rsb")
                nc.gpsimd.partition_broadcast(rsb[:], rd[:], channels=64)
                nc.gpsimd.tensor_mul(
                    UN[p][hb:hb + 64, qc * 1024:(qc + 1) * 1024],
                    ur[:], rsb[:])
            else:
                # PE broadcast of 1/D (K=1 matmul), then multiply on GpSimd
                for j in range(2):
                    bc = mm_ps.tile([128, 512], f32, tag="mm", name="bc")
                    nc.tensor.matmul(bc[:], ones32_t[:],
                                     rd[:, j * 512:(j + 1) * 512],
                                     start=True, stop=True)
                    rsb = r_pool.tile([64, 1024], f32, tag="rsb", name="rsb")
                    nc.vector.tensor_copy(rsb[:, 0:512], bc[0:64, :])
                    nc.gpsimd.tensor_mul(
                        UN[p][hb:hb + 64,
                              qc * 1024 + j * 512:qc * 1024 + j * 512 + 512],
                        ur[:, j * 512:(j + 1) * 512], rsb[:, 0:512])

        S_tiles = {0: st0, 1: st1}
        U_box = [None]

        passA_added = [False, False]
        passB_added = [False]
        fin_state = {}

        def setup_fin():
            w_pool.release()
            xt_pool.release()
            fin_state["pool"] = tc.alloc_tile_pool(name="fin", bufs=1)
            fin_state["FIN"] = [
                fin_state["pool"].tile([128, DIM], bf16, tag=f"fin{qf}",
                                       name=f"fin{qf}")
                for qf in range(16)]

        def passA(qf, of):
            FIN = fin_state["FIN"]
            ps = mm_ps.tile([128, 512], f32, tag="mm", name="pa")
            for p in range(3):
                nc.tensor.matmul(
                    ps[:], UN[p][:, qf * 128:(qf + 1) * 128],
                    WO[p][:, of * 512:(of + 1) * 512],
                    start=(p == 0), stop=(p == 2))
            nc.vector.tensor_add(
                FIN[qf][:, of * 512:(of + 1) * 512], ps[:],
                bias_t[:, of * 512:(of + 1) * 512])

        def passB_of(qf, of):
            # add pair 3 onto the resident partial and stream that
            # column-half out immediately (don't wait for the full row).
            FIN = fin_state["FIN"]
            ps = mm_ps.tile([128, 512], f32, tag="mm", name="pb")
            nc.tensor.matmul(
                ps[:], UN[3][:, qf * 128:(qf + 1) * 128],
                WO[3][:, of * 512:(of + 1) * 512],
                start=True, stop=True)
            nc.vector.tensor_add(
                FIN[qf][:, of * 512:(of + 1) * 512],
                FIN[qf][:, of * 512:(of + 1) * 512], ps[:])
            # tail out-DMAs split across both HWDGE queues; in-loop ones
            # stay off the ScalarE queue (it paces the ACTs)
            dma_eng = nc.scalar if (qf >= 8 and of == 1) else nc.sync
            dma_eng.dma_start(
                out_d.ap()[qf * 128:(qf + 1) * 128,
                           of * 512:(of + 1) * 512],
                FIN[qf][:, of * 512:(of + 1) * 512])

        def passB(qf):
            passB_of(qf, 0)
            passB_of(qf, 1)

        es_tiles = {}

        def pv_job(gp, gnow):
            """PV for step gp (retimed: late enough that a late V tile or
            a pending U drain never blocks the in-order PE queue)."""
            u, kc = divmod(gp, 16)
            p, hh, qc = units[u]
            hloc = 2 * p + hh
            es = es_tiles.pop(gp)
            if kc == 0:
                U_box[0] = u_ps.tile([65, 1024], f32, tag="u", name="u")
            U = U_box[0]
            force_v(kc)
            for j in range(2):
                nc.tensor.matmul(
                    U[:, j * 512:(j + 1) * 512],
                    VT[kc][:, hloc, 0:65],
                    es[:, j * 512:(j + 1) * 512],
                    start=(kc == 0), stop=(kc == 15))
            if kc == 15:
                # fast U drain: D row first (it gates the lazy normalize),
                # then the U rows; frees U's single psum buffer inside the
                # 3-step boundary window.
                dsb = d_pool.tile([1, 1024], f32, tag="d", name="d")
                nc.vector.tensor_copy(dsb[:], U[64:65, :])
                ur = ur_pool.tile([64, 1024], bf16, tag="ur", name="ur")
                nc.vector.tensor_copy(ur[:], U[0:64, :])
                if u >= 14:
                    # last two units: normalize eagerly on PE+DVE so the
                    # tail passB isn't gated by the GpSimd queue.
                    norm_rest(p, hh, qc, ur, dsb, on_dve=True)
                    if u == 15:
                        # keep the PE HAM clock-gate warm across the
                        # norm->passB handoff (a >3.4us PE-idle window
                        # re-throttles to K=4/8 and runs the 32 tail
                        # matmuls at half clock)
                        for _ in range(6):
                            wps = mm_ps.tile([128, 512], f32, tag="mm",
                                             name="wu2")
                            nc.tensor.matmul(wps[:], warm_t[:, 0:128],
                                             warm_t[:], start=True, stop=True)
                else:
                    add_filler(min(gnow + 10, 250),
                               lambda p=p, hh=hh, qc=qc, ur=ur, dsb=dsb:
                               norm_rest(p, hh, qc, ur, dsb))
                if u == 9 and not passA_added[0]:
                    # pairs 0-2 qc0 done: out-proj for tokens 0-1023
                    passA_added[0] = True
                    setup_fin()
                    for i, (qf, of) in enumerate(
                            (qf, of) for qf in range(8) for of in range(2)):
                        add_filler(gnow + 12 + i * 3,
                                   lambda qf=qf, of=of: passA(qf, of))
                if u == 11 and not passA_added[1]:
                    passA_added[1] = True
                    for i, (qf, of) in enumerate(
                            (qf, of) for qf in range(8, 16) for of in range(2)):
                        add_filler(gnow + 12 + i * 2,
                                   lambda qf=qf, of=of: passA(qf, of))
                if u == 13 and not passB_added[0]:
                    passB_added[0] = True
                    i = 0
                    for qf in range(8):
                        for of in range(2):
                            add_filler(gnow + 12 + i,
                                       lambda qf=qf, of=of: passB_of(qf, of))
                            i += 1

        for gidx in range(LAST_STEP + 1):
            if gidx < GTOT:
                st = S_tiles.pop(gidx)
                es = es_pool.tile([128, 1024], bf16, tag="es", name="es")
                if gidx < 2:
                    # halves: the j=0 ACT only depends on the early j=0
                    # S matmul, not on the Q chunk-1 projection
                    for j in range(2):
                        nc.scalar.activation(
                            es[:, j * 512:(j + 1) * 512],
                            st[:, j * 512:(j + 1) * 512], EXP, scale=0.125)
                else:
                    nc.scalar.activation(es[:], st[:], EXP, scale=0.125)
                es_tiles[gidx] = es
            # PV before the S lookahead: S(g+2)'s first matmul carries a
            # write-after-read wait on ACT(g)'s psum buffer, and the
            # in-order PE queue would stall on it with ready PV work
            # parked behind. Fillers AFTER S: in heavy steps (the early
            # V crunch) a 10+-matmul filler burst ahead of S would
            # starve the ACT chain instead.
            for gp in pv_at.get(gidx, ()):
                pv_job(gp, gidx)
            if gidx + 2 < GTOT:
                S_tiles[gidx + 2] = emit_S(gidx + 2)
            maybe_fill(gidx, GTOT)

        # flush remaining fillers (incl. last norms and any passA/B)
        while fillers:
            pop_filler()

        # tail: second token-half out-proj + DMA
        for qf in range(8, 16):
            passB(qf)

        fin_state["pool"].release()
        wo_pool.release()
        qkv_pool.release()

    nc.compile()
    return nc


def _get_nc():
    if "nc" not in _CACHE:
        _CACHE["nc"] = _build_nc()
    return _CACHE["nc"]


def _make_in_maps(x, w_qkv, w_out, b_out):
    bf = ml_dtypes.bfloat16

    def wslice(w, hh):
        # fc-major: [1024, 512] -> [128, 8, 512] (partition p holds
        # w[fc*128+p, :] at slot fc) -> [128, 4096]
        s = np.asarray(w[:, hh * 512:(hh + 1) * 512], np.float32)
        return np.ascontiguousarray(
            s.reshape(8, 128, 512).transpose(1, 0, 2).reshape(128, 4096)
        ).astype(bf)

    def wslice_pair(w, hh):
        # pair-major: cols [p*1024+fc*128 : +128] hold pair p's fc-chunk
        # (partition = contraction row within the chunk)
        s = np.asarray(w[:, hh * 512:(hh + 1) * 512], np.float32)
        return np.ascontiguousarray(
            s.reshape(8, 128, 4, 128).transpose(1, 2, 0, 3).reshape(128, 4096)
        ).astype(bf)

    xts = [np.ascontiguousarray(np.asarray(x[b], np.float32).T).astype(bf)
           for b in range(B)]
    wq_f = w_qkv[:, 0:1024]
    wk_f = w_qkv[:, 1024:2048]
    wv_f = w_qkv[:, 2048:3072]
    wo_f = np.asarray(w_out, np.float32)  # [1024 inner, 1024 out]
    bias_rep = np.broadcast_to(
        np.asarray(b_out, np.float32).reshape(1, DIM), (128, DIM))
    zeros = np.zeros((128, DIM), np.float32)
    in_maps = []
    for i in range(NCORES):
        b, hh = i // 2, i % 2
        wo_core = np.ascontiguousarray(
            wo_f[hh * 512:(hh + 1) * 512, :]).reshape(NPAIR, 128, DIM)
        in_maps.append({
            "xt": xts[b],
            "wq": wslice_pair(wq_f, hh),
            "wk": wslice_pair(wk_f, hh),
            "wv": wslice(wv_f, hh),
            "wo": wo_core.astype(bf),
            "bias": np.ascontiguousarray(
                (bias_rep if hh == 0 else zeros)).astype(bf),
        })
    return in_maps


def _assemble(results):
    out = np.empty((B, N, DIM), np.float32)
    for b in range(B):
        out[b] = (results[2 * b]["out"].astype(np.float32) +
                  results[2 * b + 1]["out"].astype(np.float32))
    return out


def run(x, w_qkv, w_out, b_out, trace=False):
    """Run the kernel; returns (output, BassKernelResults)."""
    from concourse.bass_utils import run_bass_kernel_spmd
    nc = _get_nc()
    in_maps = _make_in_maps(x, w_qkv, w_out, b_out)
    res = run_bass_kernel_spmd(nc, in_maps, core_ids=list(range(NCORES)),
                               trace=trace)
    return _assemble(res.results), res


def kernel(x, w_qkv, w_out, b_out):
    out, _ = run(x, w_qkv, w_out, b_out, trace=False)
    return out
